# revision 12
# baseline (speedup 1.0000x reference)
"""Causal self-attention Trainium2 kernel (B=4, T=2048, C=2048, H=16).

Sharding: 8 cores = 4 batches x 2 head-groups (8 heads each).
Each core computes, for its (batch b, head-group g):
  qkvT = Wqkv_g @ x_b^T          (bf16 matmuls, fp32 psum)
  per head: S = Q K^T (causal), A = softmax(S)  (no max subtraction --
            |S| <= ~3 for this problem), O^T = V^T A^T
  partial_b_g = (O^T)^T @ Wproj_g^T             [T, C] fp32
Host sums the two head-group partials per batch and adds b_proj.
"""

import math
import os
import numpy as np
import ml_dtypes
from contextlib import ExitStack

import concourse.bass as bass
import concourse.tile as tile
from concourse import bacc, mybir
from concourse import bass_utils

BF16 = mybir.dt.bfloat16
F32 = mybir.dt.float32
AF = mybir.ActivationFunctionType

B, T, C, H = 4, 2048, 2048, 16
D = 128          # head dim
NH = 8           # heads per core
NCC = 16         # contraction chunks of 128 over C
NTT = 16         # t tiles of 128
TCH = 512        # qkv psum free-dim chunk
NQC = T // TCH   # 4
DEPTH = int(os.environ.get("ATTN_DEPTH", "3"))  # AV software-pipeline depth


def build_program():
    nc = bacc.Bacc(
        "TRN2",
        target_bir_lowering=False,
        debug=False,
        enable_asserts=False,
        num_devices=8,
    )

    xT = nc.dram_tensor("xT", [128, NCC, T], BF16, kind="ExternalInput").ap()
    wqkv = nc.dram_tensor("wqkv", [NH, 3, 128, NCC, 128], BF16, kind="ExternalInput").ap()
    wproj = nc.dram_tensor("wproj", [2, 128, NH, 1024], BF16, kind="ExternalInput").ap()
    biasd = nc.dram_tensor("biasd", [128, 32], F32, kind="ExternalInput").ap()
    maskd = nc.dram_tensor("maskd", [128, 128], F32, kind="ExternalInput").ap()
    identd = nc.dram_tensor("identd", [128, 128], BF16, kind="ExternalInput").ap()
    partial = nc.dram_tensor("partial", [NTT, 128, C], F32, kind="ExternalOutput").ap()

    # causal-packed A^T column offsets: block j spans (16-j)*128 cols
    ATW = sum((NTT - j) * 128 for j in range(NTT))  # 17408
    atoff = [0] * NTT
    for j in range(1, NTT):
        atoff[j] = atoff[j - 1] + (NTT - (j - 1)) * 128

    with tile.TileContext(nc) as tc, ExitStack() as ctx:
        const_pool = ctx.enter_context(tc.tile_pool(name="const", bufs=1))
        xt_pool = ctx.enter_context(tc.tile_pool(name="xt", bufs=1))
        wt_pool = ctx.enter_context(tc.tile_pool(name="wt", bufs=4))
        qk_pool = ctx.enter_context(tc.tile_pool(name="qk", bufs=2))
        vt_pool = ctx.enter_context(tc.tile_pool(name="vt", bufs=1))
        v_pool = ctx.enter_context(tc.tile_pool(name="v", bufs=3))
        a_pool = ctx.enter_context(tc.tile_pool(name="a", bufs=2))
        atp_pool = ctx.enter_context(tc.tile_pool(name="atp", bufs=1))
        ots_pool = ctx.enter_context(tc.tile_pool(name="ots", bufs=2))
        wp_pool = ctx.enter_context(tc.tile_pool(name="wp", bufs=1))
        pst_pool = ctx.enter_context(tc.tile_pool(name="pst", bufs=12))
        st_pool = ctx.enter_context(tc.tile_pool(name="st", bufs=4))
        ev_pool = ctx.enter_context(tc.tile_pool(name="ev", bufs=3))
        dram_pool = ctx.enter_context(tc.tile_pool(name="scr", bufs=2, space="DRAM"))
        ps_qkv = ctx.enter_context(tc.tile_pool(name="psq", bufs=2, space="PSUM"))
        ps_s = ctx.enter_context(tc.tile_pool(name="pss", bufs=2, space="PSUM"))
        ps_o = ctx.enter_context(tc.tile_pool(name="pso", bufs=1, space="PSUM"))

        bias_sb = const_pool.tile([128, 32], F32, tag="bias")
        nc.sync.dma_start(bias_sb[:], biasd[:])
        mask_sb = const_pool.tile([128, 128], F32, tag="mask")
        nc.sync.dma_start(mask_sb[:], maskd[:])
        ident_sb = const_pool.tile([128, 128], BF16, tag="ident")
        nc.sync.dma_start(ident_sb[:], identd[:])
        xt_sb = xt_pool.tile([128, NCC, T], BF16, tag="xt")
        for cc in range(NCC):
            nc.sync.dma_start(xt_sb[:, cc, :], xT[:, cc, :])

        oT_scr = dram_pool.tile([NH, 128, T], BF16, tag="otd")

        qkt = {}    # h -> (qT, kT)
        vsb = {}    # h -> v tile
        a_scr = {}  # h -> DRAM A tile
        atp = {}    # h -> packed A^T tile

        def qkv_units(h):
            units = []
            wts = {}

            def load_w(mat):
                wt = wt_pool.tile([128, NCC, 128], BF16, tag="wt")
                nc.sync.dma_start(wt[:], wqkv[h, mat])
                wts[mat] = wt

            qT = qk_pool.tile([128, T], BF16, tag="qT")
            kT = qk_pool.tile([128, T], BF16, tag="kT")
            vT = vt_pool.tile([128, T], BF16, tag="vT")
            qkt[h] = (qT, kT)

            def mm_group(mat, dst, tch):
                def emit():
                    if tch == 0:
                        load_w(mat)
                    wt = wts[mat]
                    ps = ps_qkv.tile([128, TCH], F32, tag="pq")
                    for cc in range(NCC):
                        nc.tensor.matmul(
                            ps[:],
                            lhsT=wt[:, cc, :],
                            rhs=xt_sb[:, cc, tch * TCH:(tch + 1) * TCH],
                            start=(cc == 0),
                            stop=(cc == NCC - 1),
                        )
                    nc.scalar.activation(
                        dst[:, tch * TCH:(tch + 1) * TCH], ps[:],
                        AF.Identity, bias=bias_sb[:, mat * 8 + h:mat * 8 + h + 1],
                    )
                return emit

            for mat, dst in ((0, qT), (1, kT), (2, vT)):
                for tch in range(NQC):
                    units.append(mm_group(mat, dst, tch))

            def vtrans():
                v_sb = v_pool.tile([128, NTT, 128], BF16, tag="v")
                vsb[h] = v_sb
                for j in range(NTT):
                    pt = ps_qkv.tile([128, 128], BF16, tag="pq")
                    nc.tensor.transpose(
                        pt[:], vT[:, j * 128:(j + 1) * 128], ident_sb[:]
                    )
                    if j % 2 == 0:
                        nc.vector.tensor_copy(v_sb[:, j, :], pt[:])
                    else:
                        nc.scalar.copy(v_sb[:, j, :], pt[:])
            units.append(vtrans)
            return units

        def front_row(f, i):
            qT, kT = qkt[f]
            A_scr = a_scr[f]
            ncol = (i + 1) * 128
            nch = (ncol + TCH - 1) // TCH
            A_t = a_pool.tile([128, T], BF16, tag="A")
            ls = st_pool.tile([128, 8], F32, tag="ls")
            for c in range(nch):
                c0 = c * TCH
                cw = min(TCH, ncol - c0)
                ps = ps_s.tile([128, TCH], F32, tag="ps")
                nc.tensor.matmul(
                    ps[:, :cw],
                    lhsT=qT[:, i * 128:(i + 1) * 128],
                    rhs=kT[:, c0:c0 + cw],
                    start=True, stop=True,
                )
                if c == nch - 1:
                    nc.vector.tensor_add(
                        ps[:, cw - 128:cw], ps[:, cw - 128:cw], mask_sb[:]
                    )
                nc.scalar.activation(
                    A_t[:, c0:c0 + cw], ps[:, :cw], AF.Exp,
                    accum_out=ls[:, c:c + 1],
                )
            linv = st_pool.tile([128, 1], F32, tag="linv")
            if nch > 1:
                lsum = st_pool.tile([128, 1], F32, tag="lsum")
                nc.vector.reduce_sum(lsum[:], ls[:, :nch], axis=mybir.AxisListType.X)
                nc.vector.reciprocal(linv[:], lsum[:])
            else:
                nc.vector.reciprocal(linv[:], ls[:, 0:1])
            nc.vector.tensor_scalar_mul(A_t[:, :ncol], A_t[:, :ncol], linv[:])
            nc.sync.dma_start(
                A_scr[i * 128:(i + 1) * 128, 0:ncol], A_t[:, :ncol]
            )

        def at_loads(f):
            A_scr = a_scr[f]
            ATp = atp_pool.tile([128, ATW], BF16, tag="atp")
            atp[f] = ATp
            for j in range(NTT):
                w = (NTT - j) * 128
                nc.sync.dma_start(
                    ATp[:, atoff[j]:atoff[j] + w],
                    A_scr[j * 128:T, j * 128:(j + 1) * 128],
                    transpose=True,
                )

        def av_back(f):
            ATp = atp.pop(f)
            v_sb = vsb.pop(f)
            po = ps_o.tile([128, T], F32, tag="po")
            for j in range(NTT):
                w = (NTT - j) * 128
                # psum-bank-aligned output chunks (bank = 512 fp32 cols)
                cuts = [0]
                first = (-(j * 128)) % TCH
                if 0 < first < w:
                    cuts.append(first)
                c = cuts[-1] + TCH
                while c < w:
                    cuts.append(c)
                    c += TCH
                cuts.append(w)
                for c0, c1 in zip(cuts[:-1], cuts[1:]):
                    nc.tensor.matmul(
                        po[:, j * 128 + c0:j * 128 + c1],
                        lhsT=v_sb[:, j, :],
                        rhs=ATp[:, atoff[j] + c0:atoff[j] + c1],
                        start=(j == 0),
                        stop=(j == NTT - 1 and c1 == w),
                        skip_group_check=True,
                    )
            ots = ots_pool.tile([128, T], BF16, tag="ots")
            nc.vector.tensor_copy(ots[:, 0:1024], po[:, 0:1024])
            nc.scalar.copy(ots[:, 1024:2048], po[:, 1024:2048])
            nc.sync.dma_start(oT_scr[f], ots[:])

        for h in range(NH + 2):
            if h >= 2:
                at_loads(h - 2)
            ua = qkv_units(h) if h < NH else []
            if 1 <= h <= NH:
                f = h - 1
                a_scr[f] = dram_pool.tile([T, T], BF16, tag="ad", name=f"ascr{f}")
                for r in range(NTT):
                    front_row(f, r)
                    if r < len(ua):
                        ua[r]()
                for k in range(NTT, len(ua)):
                    ua[k]()
            else:
                for u in ua:
                    u()
            if h >= 2:
                av_back(h - 2)

        # ---- phase 3: output projection (partial, no bias) ----
        for hf in range(2):
            wp_t = wp_pool.tile([128, NH, 1024], BF16, tag="wp")
            nc.sync.dma_start(wp_t[:], wproj[hf])
            for tt in range(NTT):
                psts = []
                for hi in range(NH):
                    pt = pst_pool.tile([128, 128], BF16, tag="pst")
                    nc.sync.dma_start(
                        pt[:], oT_scr[hi][:, tt * 128:(tt + 1) * 128]
                    )
                    psts.append(pt)
                for q in range(2):
                    ps = ps_s.tile([128, TCH], F32, tag="ps")
                    for hi in range(NH):
                        nc.tensor.matmul(
                            ps[:],
                            lhsT=psts[hi][:],
                            rhs=wp_t[:, hi, q * TCH:(q + 1) * TCH],
                            start=(hi == 0),
                            stop=(hi == NH - 1),
                        )
                    ev = ev_pool.tile([128, TCH], F32, tag="ev")
                    if q == 0:
                        nc.vector.tensor_copy(ev[:], ps[:])
                    else:
                        nc.scalar.copy(ev[:], ps[:])
                    nc.sync.dma_start(
                        partial[tt][:, hf * 1024 + q * TCH:hf * 1024 + (q + 1) * TCH],
                        ev[:],
                    )

    nc.compile()
    return nc


_NC = None


def _get_nc():
    global _NC
    if _NC is None:
        _NC = build_program()
    return _NC


def make_in_maps(x, w_qkv, b_qkv, w_proj, b_proj):
    bf = ml_dtypes.bfloat16
    s = 1.0 / math.sqrt(D)
    mask = np.where(
        np.arange(128)[None, :] <= np.arange(128)[:, None], 0.0, -1e30
    ).astype(np.float32)

    xTs = []
    for b in range(B):
        xt = np.ascontiguousarray(x[b].T).reshape(NCC, 128, T).transpose(1, 0, 2)
        xTs.append(np.ascontiguousarray(xt).astype(bf))

    in_maps = []
    for core in range(8):
        b, g = core // 2, core % 2
        # wqkv packed per (head, mat): [h, mat, p, cc, 128], q pre-scaled by s
        wq_arr = np.empty((NH, 3, 128, NCC, 128), np.float32)
        bias = np.zeros((128, 32), np.float32)
        for hi in range(NH):
            hgl = g * NH + hi
            wq = w_qkv[hgl * D:(hgl + 1) * D, :] * s          # [D, C]
            wk = w_qkv[C + hgl * D:C + (hgl + 1) * D, :]
            wv = w_qkv[2 * C + hgl * D:2 * C + (hgl + 1) * D, :]
            for mat, wm in ((0, wq), (1, wk), (2, wv)):
                # wm.T [C, 128] -> [p, cc, 128]
                wq_arr[hi, mat] = wm.T.reshape(NCC, 128, 128).transpose(1, 0, 2)
            bias[:, hi] = b_qkv[hgl * D:(hgl + 1) * D] * s
            bias[:, 8 + hi] = b_qkv[C + hgl * D:C + (hgl + 1) * D]
            bias[:, 16 + hi] = b_qkv[2 * C + hgl * D:2 * C + (hgl + 1) * D]
        wp = w_proj.T[g * 1024:(g + 1) * 1024, :]             # [1024, C]
        # [2(half), p, hi, 1024]
        wp_arr = np.ascontiguousarray(
            wp.reshape(NH, 128, 2, 1024).transpose(2, 1, 0, 3)
        ).astype(bf)
        in_maps.append({
            "xT": xTs[b],
            "wqkv": wq_arr.astype(bf),
            "wproj": wp_arr,
            "biasd": bias,
            "maskd": mask,
            "identd": np.eye(128, dtype=np.float32).astype(bf),
        })
    return in_maps


def run_cores(in_maps, trace=False, **kw):
    nc = _get_nc()
    if trace:
        # NTFF profiling needs the antenv.axon_hooks shim in this image.
        import sys, types
        if "antenv.axon_hooks" not in sys.modules:
            from trn_agent_boot.trn_boot import _ntff_profile_via_ctypes
            hook = _ntff_profile_via_ctypes("/opt/axon/libaxon_pjrt.so")
            mod = types.ModuleType("antenv.axon_hooks")
            mod.get_axon_ntff_profile_hook = lambda: hook
            sys.modules["antenv.axon_hooks"] = mod
    return bass_utils.run_bass_kernel_spmd(
        nc, in_maps, core_ids=list(range(8)), trace=trace, **kw
    )


def kernel(x, w_qkv, b_qkv, w_proj, b_proj):
    x = np.asarray(x, np.float32)
    w_qkv = np.asarray(w_qkv, np.float32)
    b_qkv = np.asarray(b_qkv, np.float32)
    w_proj = np.asarray(w_proj, np.float32)
    b_proj = np.asarray(b_proj, np.float32)

    in_maps = make_in_maps(x, w_qkv, b_qkv, w_proj, b_proj)
    res = run_cores(in_maps, trace=False)
    out = np.empty((B, T, C), np.float32)
    for b in range(B):
        p0 = res.results[2 * b]["partial"].reshape(T, C)
        p1 = res.results[2 * b + 1]["partial"].reshape(T, C)
        out[b] = p0 + p1 + b_proj
    return out
